# revision 95
# baseline (speedup 1.0000x reference)
"""Trainium2 Bass kernel for fused self-attention (nn_Attention).

Reference computes (only q is used; k/v inputs are dead):
    qkv = q @ in_w.T + qkv_bias ; qp,kp,vp = split(qkv)
    per head: softmax(qp @ kp.T / sqrt(hd)) @ vp
    net = concat_heads @ out_w.T + out_b

Sharding: tensor-parallel over heads. 16 heads / 8 cores = 2 heads/core.
Each core projects q against its 2-head slice of in_w, runs attention for
its (2 batch x 2 head) pairs, and computes a partial output projection
against its 128 columns of out_w. Host sums the 8 partials.

v3 design (cost-model driven):
  - ACT(exp) is the hard floor: 16.8M exps/core at 1 elem/cycle/partition
    -> ~135us busy. Everything else is tucked under it.
  - PE work cut with fp8e4m3 DoubleRow matmuls (cost = out_cols/2 cycles).
    Precision scheme (emulated end-to-end rel err ~1.7e-2 < 2e-2):
      * q is host-split q = q_hi + q_lo (both fp8; q_lo lives in fp8's
        subnormal range, capturing ~97% of the hi-quantization residual).
      * w (QK and V slices) host-split w*128 = w_hi + w_lo (the 2^7 scale
        keeps both parts out of fp8 subnormals; de-scaled by 1/128 on the
        PSUM->SBUF copies). Projections = w_hi@q_hi + w_hi@q_lo + w_lo@q_hi
        (12 DoubleRow ops per 512-token chunk) -> ~0.15% error.
      * scores: K is re-split into fp8 (K_hi, K_lo) on the copy-out; the
        two DoubleRow k-half slots contract (K_hi + K_lo) @ Q8 exactly, so
        only the single Q-side fp8 quantization (one DR per head-tile)
        contributes error (~1.4e-2).
      * PV and out-proj stay f16.
  - V path: direct V^T DoubleRow projection -> [token, dim] PSUM tile, one
    strided de-scaling copy into v_sb; no PE transposes.
  - pv accumulators are copied PSUM->SBUF right after each chunk so the 2
    psum banks recycle; normalize (recip -> gpsimd broadcast -> mult) runs
    from SBUF off the critical path.
  - PE p-state: sem-blocked idle resets the clock ramp, so a calibrated
    run of warm matmuls bridges the initial DMA wait and hands the PE to
    the first projection already at full clock.
  - Weave: QK/V^T/proj units are split into <=~450ns parts assigned to
    explicit (b, ch, tt) emission slots ordered by streaming deadlines.
  - Tail: per-(ot, chunk) proj units; the last chunk's stage copies are
    split between DVE and the then-idle ACT engine, with psum tiles drawn
    from two pools to deepen the pipeline.
"""

import sys

for p in ("/opt/trn_rl_repo", "/root/.axon_site/_ro/trn_rl_repo"):
    if p not in sys.path:
        sys.path.append(p)

import numpy as np

B, S, D, H = 2, 2048, 1024, 16
BS = B * S  # 4096
HD = 64  # head dim
NCORES = 8
HPC = H // NCORES  # 2 heads per core -> 128 o-dims per core
WSC = 128.0  # fp8 weight-split scale (2^7)
N_WARM = 24  # PE clock-ramp bridge matmuls
EXACT_TT = (3, 6, 9, 12, 14, 15)  # score t-tiles given the K@Q_lo correction

_COMPILED = {}


def _build():
    import concourse.bass as bass  # noqa: F401
    import concourse.mybir as mybir
    import concourse.tile as tile
    from concourse import bacc

    f16 = mybir.dt.float16
    f32 = mybir.dt.float32
    f8 = mybir.dt.float8e4
    AF = mybir.ActivationFunctionType
    DR = mybir.MatmulPerfMode.DoubleRow

    nc = bacc.Bacc("TRN2", target_bir_lowering=False, debug=False,
                   num_devices=NCORES)

    q8hi_d = nc.declare_dram_parameter("q8hi", [D, BS], f8, isOutput=False)
    q8lo_d = nc.declare_dram_parameter("q8lo", [D, BS], f8, isOutput=False)
    # weights host-prearranged to [128, n*cols] partition-major layouts so
    # the DMA inner runs are >=512B (short runs pay 2x in the DMA engine)
    w8hi_d = nc.declare_dram_parameter("w8hi", [128, 2048], f8, isOutput=False)
    w8lo_d = nc.declare_dram_parameter("w8lo", [128, 2048], f8, isOutput=False)
    wv8hi_d = nc.declare_dram_parameter("wv8hi", [128, 1024], f8,
                                        isOutput=False)
    wv8lo_d = nc.declare_dram_parameter("wv8lo", [128, 1024], f8,
                                        isOutput=False)
    w2_d = nc.declare_dram_parameter("w2", [128, D], f16, isOutput=False)
    qkb_d = nc.declare_dram_parameter("qkb", [128, 2], f32, isOutput=False)
    vb_d = nc.declare_dram_parameter("vb", [1, 128], f16, isOutput=False)
    out_d = nc.declare_dram_parameter("partial", [D, BS], f16, isOutput=True)

    with tile.TileContext(nc) as tc:
        with (
            tc.tile_pool(name="persist", bufs=1) as persist,
            tc.tile_pool(name="exp", bufs=5) as exp_pool,
            tc.tile_pool(name="outT", bufs=2) as outT_pool,
            tc.tile_pool(name="pvsb", bufs=6) as pvsb_pool,
            tc.tile_pool(name="recip", bufs=6) as recip_pool,
            tc.tile_pool(name="rep", bufs=6) as rep_pool,
            tc.tile_pool(name="stage", bufs=6) as stage_pool,
            tc.tile_pool(name="ktmp", bufs=3) as ktmp_pool,
        ):
            # ---- resident SBUF tensors ----
            q8hi_sb = persist.tile([128, 8, BS], f8)    # 32KB/part
            q8lo_sb = persist.tile([128, 8, BS], f8)    # 32KB/part
            w8hi_sb = persist.tile([128, 8, 256], f8)
            w8lo_sb = persist.tile([128, 8, 256], f8)
            wv8hi_sb = persist.tile([128, 8, 128], f8)
            wv8lo_sb = persist.tile([128, 8, 128], f8)
            w2_sb = persist.tile([128, D], f16)
            qkb_sb = persist.tile([128, 2], f32)
            vb_sb = persist.tile([1, 128], f16)
            ones_sb = persist.tile([1, 128], f16)
            # projected Q,K in fp8: [m(Q/K), b, khalf, 2048 tok]
            #   m=0 (Q): khalf 0 and 1 both hold Q8 (the DR rhs reads both)
            #   m=1 (K): khalf 0 = K_hi, khalf 1 = K_lo (exact split pair)
            qk8_sb = persist.tile([128, 2, 2, 2, 2048], f8)  # 16KB/part
            # Q8 residual (duplicated planes) for the exact score tiles:
            # a second DR op adds K @ Q_lo there
            qlo8_sb = persist.tile([128, 2, 2, 2048], f8)    # 8KB/part
            # V^T: [token-in-tile, b, tile, head, 65]; col 64 per head = ones
            # -> the PV matmul also produces the softmax denominator (row 64)
            v_sb = persist.tile([128, B, 16, HPC, 65], f16)
            warm_sb = persist.tile([1, 8], f32)
            warm_mm = persist.tile([128, 128], f16)

            nc.vector.memset(ones_sb[:, :], 1.0)
            nc.vector.memset(v_sb[:, :, :, :, 64:65], 1.0)
            nc.vector.memset(warm_mm[:, :], 1.0)
            # zero the DR operand planes read by sc(b0,ch0,tt0) so it can
            # issue from K_hi + Q8 alone (tt0 sees K_hi-only: negligible)
            nc.vector.memset(qk8_sb[:, 0, 0, 1, 0:512], 0.0)
            nc.vector.memset(qk8_sb[:, 1, 0, 1, 0:128], 0.0)
            # force the exp ACT-table load NOW, before big DMAs occupy the
            # queues -- otherwise it gates the first real exp
            nc.vector.memset(warm_sb[:, :], 0.0)
            nc.scalar.activation(warm_sb[:, :], warm_sb[:, :], AF.Exp)

            # loads ordered by first use (the DMA engine is serial)
            qhi_t = q8hi_d.rearrange("(n p) m -> p n m", p=128)
            qlo_t = q8lo_d.rearrange("(n p) m -> p n m", p=128)
            w8hi_t = w8hi_d.rearrange("p (n m) -> p n m", n=8)
            w8lo_t = w8lo_d.rearrange("p (n m) -> p n m", n=8)
            wv8hi_t = wv8hi_d.rearrange("p (n m) -> p n m", n=8)
            wv8lo_t = wv8lo_d.rearrange("p (n m) -> p n m", n=8)
            # order matches first use: K00 part_a needs w8hi+q8hi0, part_b
            # adds q8lo0, part_c adds w8lo
            nc.sync.dma_start(w8hi_sb[:, :, :], w8hi_t[:, :, :])
            nc.sync.dma_start(q8hi_sb[:, :, 0:512], qhi_t[:, :, 0:512])
            nc.sync.dma_start(q8lo_sb[:, :, 0:512], qlo_t[:, :, 0:512])
            nc.sync.dma_start(w8lo_sb[:, :, :], w8lo_t[:, :, :])
            nc.sync.dma_start(wv8hi_sb[:, :, :], wv8hi_t[:, :, :])
            nc.sync.dma_start(wv8lo_sb[:, :, :], wv8lo_t[:, :, :])
            nc.sync.dma_start(qkb_sb[:, :], qkb_d[:, :])
            nc.sync.dma_start(vb_sb[:, :], vb_d[:, :])
            for scc in range(1, 4):
                nc.sync.dma_start(q8hi_sb[:, :, scc * 512:(scc + 1) * 512],
                                  qhi_t[:, :, scc * 512:(scc + 1) * 512])
                nc.sync.dma_start(q8lo_sb[:, :, scc * 512:(scc + 1) * 512],
                                  qlo_t[:, :, scc * 512:(scc + 1) * 512])
            nc.sync.dma_start(w2_sb[:, :], w2_d[:, :])
            for scc in range(4, 8):
                nc.sync.dma_start(q8hi_sb[:, :, scc * 512:(scc + 1) * 512],
                                  qhi_t[:, :, scc * 512:(scc + 1) * 512])
                nc.sync.dma_start(q8lo_sb[:, :, scc * 512:(scc + 1) * 512],
                                  qlo_t[:, :, scc * 512:(scc + 1) * 512])

            AluOp = mybir.AluOpType

            # ---- work-unit emitters -------------------------------------
            def qk_unit(pool, b, m, scc, nm, ktmp_on_act=False):
                """Q (m=0) or K (m=1) projection of one 512-token chunk:
                12 DoubleRow matmuls (w_hi@q_hi + w_hi@q_lo + w_lo@q_hi at
                the common 2^7 scale), then de-scale + bias + fp8 split on
                the copy-out. Parts a/b/c = 4 DR each (~430ns).
                ktmp_on_act routes the f16 de-scale copy to the Scalar
                engine -- startup only, while ACT is otherwise idle."""
                s0 = scc * 512  # token offset local to batch b
                t0 = b * 2048 + s0
                ref = {}

                def quad(w8, q8, start, stop):
                    for i in range(4):
                        nc.tensor.matmul(
                            ref["ps"][:, :],
                            w8[:, 2 * i:2 * i + 2, m * 128:(m + 1) * 128],
                            q8[:, 2 * i:2 * i + 2, t0:t0 + 512],
                            start=(start and i == 0),
                            stop=(stop and i == 3),
                            perf_mode=DR,
                        )

                def part_a():
                    ref["ps"] = pool.tile([128, 512], f32, tag="wv",
                                          name=f"qk{nm}")
                    quad(w8hi_sb, q8hi_sb, True, False)

                def part_b():
                    quad(w8hi_sb, q8lo_sb, False, False)

                def part_c():
                    quad(w8lo_sb, q8hi_sb, False, True)

                def part_d():
                    if m == 0:
                        # Qtmp(f16) -> Q8 (dup planes) and Q_lo (dup planes)
                        qt = ktmp_pool.tile([128, 512], f16, tag="kt",
                                            name=f"qt{nm}")
                        nc.vector.tensor_scalar(
                            qt[:, :], ref["ps"][:, :],
                            1.0 / WSC, qkb_sb[:, 0:1],
                            AluOp.mult, AluOp.add,
                        )
                        nc.vector.tensor_copy(qk8_sb[:, 0, b, 0, s0:s0 + 512],
                                              qt[:, :])
                        nc.vector.tensor_copy(qk8_sb[:, 0, b, 1, s0:s0 + 512],
                                              qk8_sb[:, 0, b, 0, s0:s0 + 512])
                        nc.vector.tensor_sub(qlo8_sb[:, b, 0, s0:s0 + 512],
                                             qt[:, :],
                                             qk8_sb[:, 0, b, 0, s0:s0 + 512])
                        nc.vector.tensor_copy(qlo8_sb[:, b, 1, s0:s0 + 512],
                                              qlo8_sb[:, b, 0, s0:s0 + 512])
                    else:
                        # exact split: Ktmp(f16) -> K_hi = fp8(Ktmp),
                        # K_lo = fp8(Ktmp - K_hi)
                        kt = ktmp_pool.tile([128, 512], f16, tag="kt",
                                            name=f"kt{nm}")
                        if ktmp_on_act:
                            nc.scalar.activation(
                                kt[:, :], ref["ps"][:, :], AF.Copy,
                                scale=1.0 / WSC, bias=qkb_sb[:, 1:2],
                            )
                        else:
                            nc.vector.tensor_scalar(
                                kt[:, :], ref["ps"][:, :],
                                1.0 / WSC, qkb_sb[:, 1:2],
                                AluOp.mult, AluOp.add,
                            )
                        nc.vector.tensor_copy(qk8_sb[:, 1, b, 0, s0:s0 + 512],
                                              kt[:, :])
                        nc.vector.tensor_sub(qk8_sb[:, 1, b, 1, s0:s0 + 512],
                                             kt[:, :],
                                             qk8_sb[:, 1, b, 0, s0:s0 + 512])
                return [part_a, part_b, part_c, part_d]

            def vt_unit(pool, b, st, nm, on_act=False):
                """Direct V^T projection of one 128-token tile via 12 DR
                (scaled splits) + bias ones-matmul + de-scaling copy into
                v_sb. One part (~375ns PE). on_act routes the de-scale
                copy to ACT (a pure Copy-with-scale) -- used where DVE is
                the production bottleneck and ACT is starved anyway."""
                t0 = b * 2048 + st * 128

                def emit():
                    ps = pool.tile([128, 2, 64], f32, tag="wv", name=f"vt{nm}")
                    terms = ((q8hi_sb, wv8hi_sb), (q8lo_sb, wv8hi_sb),
                             (q8hi_sb, wv8lo_sb))
                    for ti, (q8, wv8) in enumerate(terms):
                        for i in range(4):
                            nc.tensor.matmul(
                                ps[:, :, :],
                                q8[:, 2 * i:2 * i + 2, t0:t0 + 128],
                                wv8[:, 2 * i:2 * i + 2, :],
                                start=(ti == 0 and i == 0), stop=False,
                                perf_mode=DR,
                            )
                    nc.tensor.matmul(  # += ones.T @ (vb*128) at psum scale
                        ps[:, :, :],
                        ones_sb[0:1, :],
                        vb_sb[0:1, :],
                        start=False, stop=True,
                    )
                    # de-scale into v_sb (dst stride 65 skips the ones col)
                    if on_act:
                        nc.scalar.activation(v_sb[:, b, st, :, 0:64],
                                             ps[:, :, :], AF.Copy,
                                             scale=1.0 / WSC)
                    else:
                        nc.vector.tensor_scalar_mul(v_sb[:, b, st, :, 0:64],
                                                    ps[:, :, :], 1.0 / WSC)
                return [emit]

            def proj_unit(pool, b, ot, ch, outT_sb, nm, on_act=False):
                """Output projection for one (128 out-dims, 512 tokens)
                block: 1 matmul -> stage copy (DVE, or ACT when it has
                slack) -> DMA store."""
                def emit():
                    ps = pool.tile([128, 512], f32, tag=pool_tag[id(pool)],
                                   name=f"pj{nm}")
                    nc.tensor.matmul(
                        ps[:, :],
                        w2_sb[:, ot * 128:(ot + 1) * 128],
                        outT_sb[:, ch, :],
                        start=True, stop=True,
                    )
                    stage = stage_pool.tile([128, 512], f16, tag="st",
                                            name=f"st{nm}")
                    if on_act:
                        nc.scalar.copy(stage[:, :], ps[:, :])
                    else:
                        nc.vector.tensor_copy(stage[:, :], ps[:, :])
                    nc.sync.dma_start(
                        out_d[ot * 128:(ot + 1) * 128,
                              b * 2048 + ch * 512:b * 2048 + (ch + 1) * 512],
                        stage[:, :],
                    )
                return [emit]

            with tc.tile_pool(name="wvps", bufs=2, space="PSUM") as W:
                pool_tag = {id(W): "wv"}
                # PE clock-ramp bridge: keep the PE continuously busy from
                # t~0.8us until the first q8 chunk lands (~5.8us), so the
                # first projection runs at full clock (idle resets the ramp)
                wps = W.tile([128, 128], f32, tag="wv", name="wps")
                for i in range(N_WARM):
                    nc.tensor.matmul(wps[:, :], warm_mm[:, :], warm_mm[:, :],
                                     start=True, stop=True)

                # ---- phase 1: minimal solid pre-work, with the copy chain
                # hand-ordered so sc(tt0)'s inputs (K_hi, K_lo, Q8 planes)
                # complete as early as possible; Q_lo (only needed by the
                # first EXACT tile, tt=1) trails ----
                kps = W.tile([128, 512], f32, tag="wv", name="qkk00")
                qps = W.tile([128, 512], f32, tag="wv", name="qkq00")
                for m, ps in ((1, kps), (0, qps)):
                    for ti, (q8, w8) in enumerate(
                            ((q8hi_sb, w8hi_sb), (q8lo_sb, w8hi_sb),
                             (q8hi_sb, w8lo_sb))):
                        for i in range(4):
                            nc.tensor.matmul(
                                ps[:, :],
                                w8[:, 2 * i:2 * i + 2,
                                   m * 128:(m + 1) * 128],
                                q8[:, 2 * i:2 * i + 2, 0:512],
                                start=(ti == 0 and i == 0),
                                stop=(ti == 2 and i == 3),
                                perf_mode=DR,
                            )
                # sc(b0,ch0,tt0) needs only K_hi + the Q8 plane: its other
                # DR operand planes were zero-memset above, and the dup /
                # K_lo / Q_lo writes are emitted AFTER sc0 (WAR-ordered
                # behind its read) just inside the psum-pool block below
                kt0 = ktmp_pool.tile([128, 512], f16, tag="kt", name="kt00")
                nc.vector.tensor_scalar(kt0[:, :], kps[:, :],
                                        1.0 / WSC, qkb_sb[:, 1:2],
                                        AluOp.mult, AluOp.add)
                nc.vector.tensor_copy(qk8_sb[:, 1, 0, 0, 0:512], kt0[:, :])
                nc.vector.tensor_scalar(qk8_sb[:, 0, 0, 0, 0:512], qps[:, :],
                                        1.0 / WSC, qkb_sb[:, 0:1],
                                        AluOp.mult, AluOp.add)
                for part in vt_unit(W, 0, 0, "v00"):
                    part()

                # ---- weave schedule -------------------------------------
                sched = {}

                def assign(slots, parts):
                    assert len(slots) >= len(parts), (len(slots), len(parts))
                    for s, p in zip(slots, parts):
                        sched.setdefault(s, []).append(p)

                def qk_slots(b, ch, t1, t2, t3, t4):
                    return [(b, ch, t1), (b, ch, t2), (b, ch, t3), (b, ch, t4)]

                # b0 ch0: K(scc1-3) ahead of score deadlines (tt=4*scc),
                # V^T(st1-15) ahead of pv deadlines (tt=st)
                assign(qk_slots(0, 0, 1, 1, 2, 3), qk_unit(W, 0, 1, 1, "k01"))
                assign(qk_slots(0, 0, 5, 5, 6, 7), qk_unit(W, 0, 1, 2, "k02"))
                assign(qk_slots(0, 0, 9, 9, 10, 11), qk_unit(W, 0, 1, 3, "k03"))
                for st in range(1, 16):
                    assign([(0, 0, st)], vt_unit(W, 0, st, f"v0{st}"))
                # Q chunks for b0 ch1-3 (due at the start of their chunk)
                assign(qk_slots(0, 0, 13, 13, 14, 15), qk_unit(W, 0, 0, 1, "q01"))
                assign(qk_slots(0, 1, 1, 1, 2, 3), qk_unit(W, 0, 0, 2, "q02"))
                assign(qk_slots(0, 2, 1, 1, 2, 3), qk_unit(W, 0, 0, 3, "q03"))
                # b1 prep spread across b0 ch2/ch3
                assign(qk_slots(0, 2, 5, 5, 6, 7), qk_unit(W, 1, 1, 0, "k10"))
                assign(qk_slots(0, 2, 9, 9, 10, 11), qk_unit(W, 1, 0, 0, "q10"))
                for st in range(0, 4):
                    assign([(0, 2, 12 + st)], vt_unit(W, 1, st, f"v1{st}"))
                assign(qk_slots(0, 3, 1, 1, 2, 3), qk_unit(W, 1, 1, 1, "k11"))
                for st in range(4, 8):
                    assign([(0, 3, 4 + st)], vt_unit(W, 1, st, f"v1{st}"))
                # b1 ch0: K(scc2,3) due tt 8,12; V^T(st8-15) due tt 8-15
                assign(qk_slots(1, 0, 1, 1, 2, 3), qk_unit(W, 1, 1, 2, "k12"))
                assign(qk_slots(1, 0, 5, 5, 6, 7), qk_unit(W, 1, 1, 3, "k13"))
                for st in range(8, 16):
                    assign([(1, 0, st)], vt_unit(W, 1, st, f"v1{st}"))
                assign(qk_slots(1, 0, 9, 9, 10, 11), qk_unit(W, 1, 0, 1, "q11"))
                assign(qk_slots(1, 1, 1, 1, 2, 3), qk_unit(W, 1, 0, 2, "q12"))
                assign(qk_slots(1, 2, 1, 1, 2, 3), qk_unit(W, 1, 0, 3, "q13"))

                # proj slots per just-finished global chunk g = 4*b + ch
                PROJ_SLOTS = {
                    0: [(0, 1, t) for t in range(6, 14)],
                    1: [(0, 3, t) for t in (4, 5, 6, 7, 12, 13, 14, 15)],
                    2: [(1, 1, t) for t in range(4, 12)],
                    3: [(1, 1, t) for t in range(12, 16)]
                       + [(1, 2, t) for t in range(4, 8)],
                    4: [(1, 2, t) for t in range(8, 16)],
                    5: [(1, 3, t) for t in range(1, 9)],
                    6: [(1, 3, t) for t in range(9, 16)] + [(1, 3, 15)],
                }

                # ---- attention + normalize + woven projections ----------
                # pv runs TWO iterations behind exp so the PE never waits
                # on an in-flight exp; each chunk's last two pv tiles, its
                # normalize, and its proj assignment are deferred into the
                # next chunk's first iterations (so the in-order PE queue
                # never blocks on the chunk's final exp at the boundary)
                outT_tiles = {}

                def finish_chunk(pend, tt_step):
                    fb, fch, fpv, felog, foutT = pend
                    if tt_step == 0:
                        pe, ptt = felog[14]
                        for h in range(HPC):
                            nc.tensor.matmul(
                                fpv[h][:, :], v_sb[:, fb, 14, h, :],
                                pe[:, h, :], start=False, stop=False)
                        return
                    pe, ptt = felog[15]
                    for h in range(HPC):
                        nc.tensor.matmul(
                            fpv[h][:, :], v_sb[:, fb, 15, h, :],
                            pe[:, h, :], start=False, stop=True)
                    for h in range(HPC):
                        pvs = pvsb_pool.tile([65, 512], f32, tag="pvs",
                                             name=f"pvs{fb}{fch}{h}")
                        nc.vector.tensor_copy(pvs[:, :], fpv[h][:, :])
                        recip = recip_pool.tile([1, 512], f32, tag="rc",
                                                name=f"rc{fb}{fch}{h}")
                        nc.vector.reciprocal(recip[:, :], pvs[64:65, :])
                        rep = rep_pool.tile([64, 512], f32, tag="rp",
                                            name=f"rp{fb}{fch}{h}")
                        nc.gpsimd.partition_broadcast(rep[:, :], recip[:, :])
                        nc.vector.tensor_mul(
                            foutT[h * 64:(h + 1) * 64, fch, :],
                            pvs[0:64, :], rep[:, :])
                    g = 4 * fb + fch
                    if g in PROJ_SLOTS:
                        parts = []
                        for ot in range(8):
                            parts += proj_unit(W, fb, ot, fch, foutT,
                                               f"p{fb}{fch}_{ot}")
                        assign(PROJ_SLOTS[g], parts)

                with tc.tile_pool(name="scps", bufs=2, space="PSUM") as scps, \
                     tc.tile_pool(name="pvps", bufs=2, space="PSUM") as pvps:
                    pool_tag[id(pvps)] = "pv"
                    # sc(b0,ch0,tt0) emitted FIRST: K_hi @ Q8 only (other
                    # planes zeroed); the dup / K_lo / Q_lo writes follow
                    # and are WAR-ordered behind its read
                    sc0 = scps.tile([128, 2, 512], f32, tag="sc",
                                    name="sc0_0_0")
                    for h in range(HPC):
                        nc.tensor.matmul(
                            sc0[:, h, :],
                            qk8_sb[h * 64:h * 64 + 64, 1, 0, :, 0:128],
                            qk8_sb[h * 64:h * 64 + 64, 0, 0, :, 0:512],
                            start=True, stop=True, perf_mode=DR,
                        )
                    e0 = exp_pool.tile([128, 2, 512], f16, tag="exp",
                                       name="e0_0_0")
                    nc.scalar.activation(e0[:, :, :], sc0[:, :, :],
                                         AF.Exp, scale=0.125)
                    nc.vector.tensor_copy(qk8_sb[:, 0, 0, 1, 0:512],
                                          qk8_sb[:, 0, 0, 0, 0:512])
                    nc.vector.tensor_sub(qk8_sb[:, 1, 0, 1, 0:512],
                                         kt0[:, :],
                                         qk8_sb[:, 1, 0, 0, 0:512])
                    qt0 = ktmp_pool.tile([128, 512], f16, tag="kt",
                                         name="qt00")
                    nc.vector.tensor_scalar(qt0[:, :], qps[:, :],
                                            1.0 / WSC, qkb_sb[:, 0:1],
                                            AluOp.mult, AluOp.add)
                    nc.vector.tensor_sub(qlo8_sb[:, 0, 0, 0:512], qt0[:, :],
                                         qk8_sb[:, 0, 0, 0, 0:512])
                    nc.vector.tensor_copy(qlo8_sb[:, 0, 1, 0:512],
                                          qlo8_sb[:, 0, 0, 0:512])
                    pending = None
                    for b in range(B):
                        outT_sb = outT_pool.tile([128, 4, 512], f16,
                                                 tag="outT", name=f"outT{b}")
                        outT_tiles[b] = outT_sb
                        for ch in range(4):
                            s0 = ch * 512
                            pv = None
                            elog = []
                            for tt in range(16):
                                if b == 0 and ch == 0 and tt == 0:
                                    elog.append((e0, 0))
                                    continue
                                t0 = tt * 128
                                sc = scps.tile([128, 2, 512], f32, tag="sc",
                                               name=f"sc{b}_{ch}_{tt}")
                                # EXACT_TT tiles get the K @ Q_lo correction
                                # -> exact scores there; the rest carry only
                                # the single Q8 quantization
                                exact = tt in EXACT_TT
                                for h in range(HPC):
                                    lo = h * 64
                                    hi = lo + 64
                                    nc.tensor.matmul(
                                        sc[:, h, :],
                                        qk8_sb[lo:hi, 1, b, :, t0:t0 + 128],
                                        qk8_sb[lo:hi, 0, b, :, s0:s0 + 512],
                                        start=True, stop=not exact,
                                        perf_mode=DR,
                                    )
                                    if exact:
                                        nc.tensor.matmul(
                                            sc[:, h, :],
                                            qk8_sb[lo:hi, 1, b, :,
                                                   t0:t0 + 128],
                                            qlo8_sb[lo:hi, b, :,
                                                    s0:s0 + 512],
                                            start=False, stop=True,
                                            perf_mode=DR,
                                        )
                                for u in sched.get((b, ch, tt), ()):
                                    u()
                                if tt < 2:
                                    if pending is not None:
                                        finish_chunk(pending, tt)
                                        if tt == 1:
                                            pending = None
                                else:
                                    if tt == 2:
                                        pv = [pvps.tile([65, 512], f32,
                                                        tag="pv",
                                                        name=f"pv{b}{ch}{h}")
                                              for h in range(HPC)]
                                    pe, ptt = elog[tt - 2]
                                    for h in range(HPC):
                                        nc.tensor.matmul(
                                            pv[h][:, :],
                                            v_sb[:, b, ptt, h, :],
                                            pe[:, h, :],
                                            start=(ptt == 0), stop=False,
                                        )
                                    if b == 1 and ch == 3 and tt == 15:
                                        # last chunk: pull pv(14) into the
                                        # loop (overlaps exp 15) so the
                                        # tail waits only on pv(15)
                                        pe14, _ = elog[14]
                                        for h in range(HPC):
                                            nc.tensor.matmul(
                                                pv[h][:, :],
                                                v_sb[:, b, 14, h, :],
                                                pe14[:, h, :],
                                                start=False, stop=False,
                                            )
                                e = exp_pool.tile([128, 2, 512], f16,
                                                  tag="exp",
                                                  name=f"e{b}_{ch}_{tt}")
                                nc.scalar.activation(e[:, :, :], sc[:, :, :],
                                                     AF.Exp, scale=0.125)
                                elog.append((e, tt))
                            pending = (b, ch, pv, elog, outT_sb)

                    # ---- tail: the final chunk's catch-up, then normalize
                    # and projection processed in TWO 256-column halves so
                    # every stage (DVE copies/recips/mults, Pool broadcasts,
                    # PE proj matmuls, stage copies, DMA) pipelines; a short
                    # warm run keeps the PE clock ramped through the wait.
                    fb, fch, fpv, felog, foutT = pending
                    for ptt in (15,):  # pv(14) already ran in the loop
                        pe, _ = felog[ptt]
                        for h in range(HPC):
                            nc.tensor.matmul(
                                fpv[h][:, :], v_sb[:, fb, ptt, h, :],
                                pe[:, h, :], start=False, stop=(ptt == 15))
                    wps2 = W.tile([128, 128], f32, tag="wv", name="wps2")
                    for i in range(40):
                        nc.tensor.matmul(wps2[:, :], warm_mm[:, :],
                                         warm_mm[:, :], start=True, stop=True)
                    for hf in range(2):
                        cs = slice(hf * 256, (hf + 1) * 256)
                        for h in range(HPC):
                            pvs = pvsb_pool.tile([65, 256], f32, tag="pvs",
                                                 name=f"pvsT{h}{hf}")
                            if h == 0:  # ACT is idle post-exp: split lanes
                                nc.scalar.copy(pvs[:, :], fpv[h][:, cs])
                            else:
                                nc.vector.tensor_copy(pvs[:, :],
                                                      fpv[h][:, cs])
                            recip = recip_pool.tile([1, 256], f32, tag="rc",
                                                    name=f"rcT{h}{hf}")
                            nc.vector.reciprocal(recip[:, :], pvs[64:65, :])
                            rep = rep_pool.tile([64, 256], f32, tag="rp",
                                                name=f"rpT{h}{hf}")
                            nc.gpsimd.partition_broadcast(rep[:, :],
                                                          recip[:, :])
                            nc.vector.tensor_mul(
                                foutT[h * 64:(h + 1) * 64, fch, cs],
                                pvs[0:64, :], rep[:, :])
                    for hf in range(2):
                        cs = slice(hf * 256, (hf + 1) * 256)
                        c0 = fb * 2048 + fch * 512 + hf * 256
                        for op in range(4):  # pairs of ot blocks
                            # each pair: one 1-bank psum tile, one copy,
                            # one DMA; psums rotate over three pools (the
                            # idle scps banks included) for a 6-deep pipe
                            pool = (scps, W, pvps)[(hf * 4 + op) % 3]
                            if pool is scps:
                                tl = scps.tile([128, 2, 512], f32,
                                               tag="sc",
                                               name=f"pjt{op}{hf}")
                                sub = lambda j: tl[:, j, 0:256]
                                pr = tl[:, :, 0:256]
                            else:
                                tl = pool.tile([128, 2, 256], f32,
                                               tag=pool_tag[id(pool)],
                                               name=f"pjt{op}{hf}")
                                sub = lambda j: tl[:, j, :]
                                pr = tl[:, :, :]
                            for j in range(2):
                                nc.tensor.matmul(
                                    sub(j),
                                    w2_sb[:, (2 * op + j) * 128:
                                          (2 * op + j + 1) * 128],
                                    foutT[:, fch, cs],
                                    start=True, stop=True,
                                )
                            stage = stage_pool.tile([128, 2, 256], f16,
                                                    tag="st",
                                                    name=f"stt{op}{hf}")
                            # DVE still owes the norm chain; give ACT the
                            # larger share of the pair copies
                            if hf * 4 + op < 5:
                                nc.scalar.copy(stage[:, :, :], pr)
                            else:
                                nc.vector.tensor_copy(stage[:, :, :], pr)
                            # spread the tail stores across three DGE
                            # queues: the SP sequencer's ~650ns/dispatch
                            # would otherwise pace them
                            # DVE-copied pairs store via the ACT DGE queue:
                            # those dispatches sit after all ACT copies in
                            # its FIFO, running parallel to SP's dispatches
                            dq = nc.scalar if hf * 4 + op >= 5 else nc.sync
                            dq.dma_start(
                                out_d[2 * op * 128:(2 * op + 2) * 128,
                                      c0:c0 + 256].rearrange(
                                          "(n p) m -> p n m", p=128),
                                stage[:, :, :],
                            )
    nc.compile()
    return nc


def _get_nc():
    if "nc" not in _COMPILED:
        _COMPILED["nc"] = _build()
    return _COMPILED["nc"]


def _prep_inputs(q, in_w, qkv_bias, out_w):
    import ml_dtypes
    f16 = np.float16
    f8 = ml_dtypes.float8_e4m3
    F = np.float32
    qT = np.ascontiguousarray(q.transpose(2, 0, 1).reshape(D, BS))
    q8hi = qT.astype(f8)
    q8lo = (qT - q8hi.astype(F)).astype(f8)

    def warr(wT, cols):  # [D, cols] -> scaled fp8 split, [128, 8*cols]
        ws = wT * WSC
        hi = ws.astype(f8)
        lo = (ws - hi.astype(F)).astype(f8)

        def pack(a):
            return np.ascontiguousarray(
                a.reshape(8, 128, cols).transpose(1, 0, 2).reshape(128, -1))
        return pack(hi), pack(lo)

    maps = []
    for c in range(NCORES):
        r = slice(128 * c, 128 * (c + 1))
        wq, wk, wv = in_w[0:D][r], in_w[D:2 * D][r], in_w[2 * D:3 * D][r]
        wqk = np.ascontiguousarray(np.concatenate([wq, wk], 0).T)  # [D, 256]
        w8hi, w8lo = warr(wqk, 256)
        wv8hi, wv8lo = warr(np.ascontiguousarray(wv.T), 128)
        qkb = np.stack([qkv_bias[0:D][r], qkv_bias[D:2 * D][r]],
                       axis=1).astype(F)  # [128, 2]
        maps.append({
            "q8hi": q8hi,
            "q8lo": q8lo,
            "w8hi": w8hi,
            "w8lo": w8lo,
            "wv8hi": wv8hi,
            "wv8lo": wv8lo,
            "w2": np.ascontiguousarray(out_w[:, r].T).astype(f16),
            "qkb": np.ascontiguousarray(qkb),
            "vb": np.ascontiguousarray(
                (qkv_bias[2 * D:3 * D][r] * WSC)[None, :]).astype(f16),
        })
    return maps


def kernel(q, k, v, in_w, qkv_bias, out_w, out_b, _trace=False):
    from concourse.bass_utils import run_bass_kernel_spmd

    q = np.asarray(q, dtype=np.float32)
    in_w = np.asarray(in_w, dtype=np.float32)
    qkv_bias = np.asarray(qkv_bias, dtype=np.float32)
    out_w = np.asarray(out_w, dtype=np.float32)
    out_b = np.asarray(out_b, dtype=np.float32)

    nc = _get_nc()
    in_maps = _prep_inputs(q, in_w, qkv_bias, out_w)

    res = run_bass_kernel_spmd(
        nc, in_maps, core_ids=list(range(NCORES)), trace=_trace,
    )
    total = np.zeros((D, BS), dtype=np.float32)
    for c in range(NCORES):
        total += res.results[c]["partial"].astype(np.float32)
    net = total.T + out_b[None, :]
    out = net.reshape(B, S, D).astype(np.float32)
    if _trace:
        return out, res
    return out


# revision 96
# speedup vs baseline: 1.0058x; 1.0058x over previous
"""Trainium2 Bass kernel for fused self-attention (nn_Attention).

Reference computes (only q is used; k/v inputs are dead):
    qkv = q @ in_w.T + qkv_bias ; qp,kp,vp = split(qkv)
    per head: softmax(qp @ kp.T / sqrt(hd)) @ vp
    net = concat_heads @ out_w.T + out_b

Sharding: tensor-parallel over heads. 16 heads / 8 cores = 2 heads/core.
Each core projects q against its 2-head slice of in_w, runs attention for
its (2 batch x 2 head) pairs, and computes a partial output projection
against its 128 columns of out_w. Host sums the 8 partials.

v3 design (cost-model driven):
  - ACT(exp) is the hard floor: 16.8M exps/core at 1 elem/cycle/partition
    -> ~135us busy. Everything else is tucked under it.
  - PE work cut with fp8e4m3 DoubleRow matmuls (cost = out_cols/2 cycles).
    Precision scheme (emulated end-to-end rel err ~1.7e-2 < 2e-2):
      * q is host-split q = q_hi + q_lo (both fp8; q_lo lives in fp8's
        subnormal range, capturing ~97% of the hi-quantization residual).
      * w (QK and V slices) host-split w*128 = w_hi + w_lo (the 2^7 scale
        keeps both parts out of fp8 subnormals; de-scaled by 1/128 on the
        PSUM->SBUF copies). Projections = w_hi@q_hi + w_hi@q_lo + w_lo@q_hi
        (12 DoubleRow ops per 512-token chunk) -> ~0.15% error.
      * scores: K is re-split into fp8 (K_hi, K_lo) on the copy-out; the
        two DoubleRow k-half slots contract (K_hi + K_lo) @ Q8 exactly, so
        only the single Q-side fp8 quantization (one DR per head-tile)
        contributes error (~1.4e-2).
      * PV and out-proj stay f16.
  - V path: direct V^T DoubleRow projection -> [token, dim] PSUM tile, one
    strided de-scaling copy into v_sb; no PE transposes.
  - pv accumulators are copied PSUM->SBUF right after each chunk so the 2
    psum banks recycle; normalize (recip -> gpsimd broadcast -> mult) runs
    from SBUF off the critical path.
  - PE p-state: sem-blocked idle resets the clock ramp, so a calibrated
    run of warm matmuls bridges the initial DMA wait and hands the PE to
    the first projection already at full clock.
  - Weave: QK/V^T/proj units are split into <=~450ns parts assigned to
    explicit (b, ch, tt) emission slots ordered by streaming deadlines.
  - Tail: per-(ot, chunk) proj units; the last chunk's stage copies are
    split between DVE and the then-idle ACT engine, with psum tiles drawn
    from two pools to deepen the pipeline.
"""

import sys

for p in ("/opt/trn_rl_repo", "/root/.axon_site/_ro/trn_rl_repo"):
    if p not in sys.path:
        sys.path.append(p)

import numpy as np

B, S, D, H = 2, 2048, 1024, 16
BS = B * S  # 4096
HD = 64  # head dim
NCORES = 8
HPC = H // NCORES  # 2 heads per core -> 128 o-dims per core
WSC = 128.0  # fp8 weight-split scale (2^7)
N_WARM = 24  # PE clock-ramp bridge matmuls
EXACT_TT = (3, 6, 9, 12, 14, 15)  # score t-tiles given the K@Q_lo correction

_COMPILED = {}


def _build():
    import concourse.bass as bass  # noqa: F401
    import concourse.mybir as mybir
    import concourse.tile as tile
    from concourse import bacc

    f16 = mybir.dt.float16
    f32 = mybir.dt.float32
    f8 = mybir.dt.float8e4
    AF = mybir.ActivationFunctionType
    DR = mybir.MatmulPerfMode.DoubleRow

    nc = bacc.Bacc("TRN2", target_bir_lowering=False, debug=False,
                   num_devices=NCORES)

    q8hi_d = nc.declare_dram_parameter("q8hi", [D, BS], f8, isOutput=False)
    q8lo_d = nc.declare_dram_parameter("q8lo", [D, BS], f8, isOutput=False)
    # weights host-prearranged to [128, n*cols] partition-major layouts so
    # the DMA inner runs are >=512B (short runs pay 2x in the DMA engine)
    w8hi_d = nc.declare_dram_parameter("w8hi", [128, 2048], f8, isOutput=False)
    w8lo_d = nc.declare_dram_parameter("w8lo", [128, 2048], f8, isOutput=False)
    wv8hi_d = nc.declare_dram_parameter("wv8hi", [128, 1024], f8,
                                        isOutput=False)
    wv8lo_d = nc.declare_dram_parameter("wv8lo", [128, 1024], f8,
                                        isOutput=False)
    w2_d = nc.declare_dram_parameter("w2", [128, D], f16, isOutput=False)
    qkb_d = nc.declare_dram_parameter("qkb", [128, 2], f32, isOutput=False)
    vb_d = nc.declare_dram_parameter("vb", [1, 128], f16, isOutput=False)
    out_d = nc.declare_dram_parameter("partial", [D, BS], f16, isOutput=True)

    with tile.TileContext(nc) as tc:
        with (
            tc.tile_pool(name="persist", bufs=1) as persist,
            tc.tile_pool(name="exp", bufs=5) as exp_pool,
            tc.tile_pool(name="outT", bufs=2) as outT_pool,
            tc.tile_pool(name="pvsb", bufs=6) as pvsb_pool,
            tc.tile_pool(name="recip", bufs=6) as recip_pool,
            tc.tile_pool(name="rep", bufs=6) as rep_pool,
            tc.tile_pool(name="stage", bufs=6) as stage_pool,
            tc.tile_pool(name="ktmp", bufs=3) as ktmp_pool,
        ):
            # ---- resident SBUF tensors ----
            q8hi_sb = persist.tile([128, 8, BS], f8)    # 32KB/part
            q8lo_sb = persist.tile([128, 8, BS], f8)    # 32KB/part
            w8hi_sb = persist.tile([128, 8, 256], f8)
            w8lo_sb = persist.tile([128, 8, 256], f8)
            wv8hi_sb = persist.tile([128, 8, 128], f8)
            wv8lo_sb = persist.tile([128, 8, 128], f8)
            w2_sb = persist.tile([128, D], f16)
            qkb_sb = persist.tile([128, 2], f32)
            vb_sb = persist.tile([1, 128], f16)
            ones_sb = persist.tile([1, 128], f16)
            # projected Q,K in fp8: [m(Q/K), b, khalf, 2048 tok]
            #   m=0 (Q): khalf 0 and 1 both hold Q8 (the DR rhs reads both)
            #   m=1 (K): khalf 0 = K_hi, khalf 1 = K_lo (exact split pair)
            qk8_sb = persist.tile([128, 2, 2, 2, 2048], f8)  # 16KB/part
            # Q8 residual (duplicated planes) for the exact score tiles:
            # a second DR op adds K @ Q_lo there
            qlo8_sb = persist.tile([128, 2, 2, 2048], f8)    # 8KB/part
            # V^T: [token-in-tile, b, tile, head, 65]; col 64 per head = ones
            # -> the PV matmul also produces the softmax denominator (row 64)
            v_sb = persist.tile([128, B, 16, HPC, 65], f16)
            warm_sb = persist.tile([1, 8], f32)
            warm_mm = persist.tile([128, 128], f16)

            nc.vector.memset(ones_sb[:, :], 1.0)
            nc.vector.memset(v_sb[:, :, :, :, 64:65], 1.0)
            nc.vector.memset(warm_mm[:, :], 1.0)
            # zero the DR operand planes read by sc(b0,ch0,tt0) so it can
            # issue from K_hi + Q8 alone (tt0 sees K_hi-only: negligible)
            nc.vector.memset(qk8_sb[:, 0, 0, 1, 0:512], 0.0)
            nc.vector.memset(qk8_sb[:, 1, 0, 1, 0:128], 0.0)
            # force the exp ACT-table load NOW, before big DMAs occupy the
            # queues -- otherwise it gates the first real exp
            nc.vector.memset(warm_sb[:, :], 0.0)
            nc.scalar.activation(warm_sb[:, :], warm_sb[:, :], AF.Exp)

            # loads ordered by first use (the DMA engine is serial)
            qhi_t = q8hi_d.rearrange("(n p) m -> p n m", p=128)
            qlo_t = q8lo_d.rearrange("(n p) m -> p n m", p=128)
            w8hi_t = w8hi_d.rearrange("p (n m) -> p n m", n=8)
            w8lo_t = w8lo_d.rearrange("p (n m) -> p n m", n=8)
            wv8hi_t = wv8hi_d.rearrange("p (n m) -> p n m", n=8)
            wv8lo_t = wv8lo_d.rearrange("p (n m) -> p n m", n=8)
            # order matches first use: K00 part_a needs w8hi+q8hi0, part_b
            # adds q8lo0, part_c adds w8lo
            nc.sync.dma_start(w8hi_sb[:, :, :], w8hi_t[:, :, :])
            nc.sync.dma_start(q8hi_sb[:, :, 0:512], qhi_t[:, :, 0:512])
            nc.sync.dma_start(q8lo_sb[:, :, 0:512], qlo_t[:, :, 0:512])
            nc.sync.dma_start(w8lo_sb[:, :, :], w8lo_t[:, :, :])
            nc.sync.dma_start(wv8hi_sb[:, :, :], wv8hi_t[:, :, :])
            nc.sync.dma_start(wv8lo_sb[:, :, :], wv8lo_t[:, :, :])
            nc.sync.dma_start(qkb_sb[:, :], qkb_d[:, :])
            nc.sync.dma_start(vb_sb[:, :], vb_d[:, :])
            for scc in range(1, 4):
                nc.sync.dma_start(q8hi_sb[:, :, scc * 512:(scc + 1) * 512],
                                  qhi_t[:, :, scc * 512:(scc + 1) * 512])
                nc.sync.dma_start(q8lo_sb[:, :, scc * 512:(scc + 1) * 512],
                                  qlo_t[:, :, scc * 512:(scc + 1) * 512])
            nc.sync.dma_start(w2_sb[:, :], w2_d[:, :])
            for scc in range(4, 8):
                nc.sync.dma_start(q8hi_sb[:, :, scc * 512:(scc + 1) * 512],
                                  qhi_t[:, :, scc * 512:(scc + 1) * 512])
                nc.sync.dma_start(q8lo_sb[:, :, scc * 512:(scc + 1) * 512],
                                  qlo_t[:, :, scc * 512:(scc + 1) * 512])

            AluOp = mybir.AluOpType

            # ---- work-unit emitters -------------------------------------
            def qk_unit(pool, b, m, scc, nm, ktmp_on_act=False,
                        skip_wlo=False):
                """Q (m=0) or K (m=1) projection of one 512-token chunk:
                12 DoubleRow matmuls (w_hi@q_hi + w_hi@q_lo + w_lo@q_hi at
                the common 2^7 scale), then de-scale + bias + fp8 split on
                the copy-out. Parts a/b/c = 4 DR each (~430ns).
                ktmp_on_act routes the f16 de-scale copy to the Scalar
                engine -- startup only, while ACT is otherwise idle."""
                s0 = scc * 512  # token offset local to batch b
                t0 = b * 2048 + s0
                ref = {}

                def quad(w8, q8, start, stop):
                    for i in range(4):
                        nc.tensor.matmul(
                            ref["ps"][:, :],
                            w8[:, 2 * i:2 * i + 2, m * 128:(m + 1) * 128],
                            q8[:, 2 * i:2 * i + 2, t0:t0 + 512],
                            start=(start and i == 0),
                            stop=(stop and i == 3),
                            perf_mode=DR,
                        )

                def part_a():
                    ref["ps"] = pool.tile([128, 512], f32, tag="wv",
                                          name=f"qk{nm}")
                    quad(w8hi_sb, q8hi_sb, True, False)

                def part_b():
                    quad(w8hi_sb, q8lo_sb, False, skip_wlo)

                def part_c():
                    quad(w8lo_sb, q8hi_sb, False, True)

                def part_d():
                    if m == 0:
                        # Qtmp(f16) -> Q8 (dup planes) and Q_lo (dup planes)
                        qt = ktmp_pool.tile([128, 512], f16, tag="kt",
                                            name=f"qt{nm}")
                        nc.vector.tensor_scalar(
                            qt[:, :], ref["ps"][:, :],
                            1.0 / WSC, qkb_sb[:, 0:1],
                            AluOp.mult, AluOp.add,
                        )
                        nc.vector.tensor_copy(qk8_sb[:, 0, b, 0, s0:s0 + 512],
                                              qt[:, :])
                        nc.vector.tensor_copy(qk8_sb[:, 0, b, 1, s0:s0 + 512],
                                              qk8_sb[:, 0, b, 0, s0:s0 + 512])
                        nc.vector.tensor_sub(qlo8_sb[:, b, 0, s0:s0 + 512],
                                             qt[:, :],
                                             qk8_sb[:, 0, b, 0, s0:s0 + 512])
                        nc.vector.tensor_copy(qlo8_sb[:, b, 1, s0:s0 + 512],
                                              qlo8_sb[:, b, 0, s0:s0 + 512])
                    else:
                        # exact split: Ktmp(f16) -> K_hi = fp8(Ktmp),
                        # K_lo = fp8(Ktmp - K_hi)
                        kt = ktmp_pool.tile([128, 512], f16, tag="kt",
                                            name=f"kt{nm}")
                        if ktmp_on_act:
                            nc.scalar.activation(
                                kt[:, :], ref["ps"][:, :], AF.Copy,
                                scale=1.0 / WSC, bias=qkb_sb[:, 1:2],
                            )
                        else:
                            nc.vector.tensor_scalar(
                                kt[:, :], ref["ps"][:, :],
                                1.0 / WSC, qkb_sb[:, 1:2],
                                AluOp.mult, AluOp.add,
                            )
                        nc.vector.tensor_copy(qk8_sb[:, 1, b, 0, s0:s0 + 512],
                                              kt[:, :])
                        nc.vector.tensor_sub(qk8_sb[:, 1, b, 1, s0:s0 + 512],
                                             kt[:, :],
                                             qk8_sb[:, 1, b, 0, s0:s0 + 512])
                if skip_wlo:
                    # spend error budget: drop the w_lo correction (the
                    # projection carries the ~2.7% w-quant error) to
                    # decongest the PE in the first-chunk window
                    return [part_a, part_b, part_d]
                return [part_a, part_b, part_c, part_d]

            def vt_unit(pool, b, st, nm, on_act=False):
                """Direct V^T projection of one 128-token tile via 12 DR
                (scaled splits) + bias ones-matmul + de-scaling copy into
                v_sb. One part (~375ns PE). on_act routes the de-scale
                copy to ACT (a pure Copy-with-scale) -- used where DVE is
                the production bottleneck and ACT is starved anyway."""
                t0 = b * 2048 + st * 128

                def emit():
                    ps = pool.tile([128, 2, 64], f32, tag="wv", name=f"vt{nm}")
                    terms = ((q8hi_sb, wv8hi_sb), (q8lo_sb, wv8hi_sb),
                             (q8hi_sb, wv8lo_sb))
                    for ti, (q8, wv8) in enumerate(terms):
                        for i in range(4):
                            nc.tensor.matmul(
                                ps[:, :, :],
                                q8[:, 2 * i:2 * i + 2, t0:t0 + 128],
                                wv8[:, 2 * i:2 * i + 2, :],
                                start=(ti == 0 and i == 0), stop=False,
                                perf_mode=DR,
                            )
                    nc.tensor.matmul(  # += ones.T @ (vb*128) at psum scale
                        ps[:, :, :],
                        ones_sb[0:1, :],
                        vb_sb[0:1, :],
                        start=False, stop=True,
                    )
                    # de-scale into v_sb (dst stride 65 skips the ones col)
                    if on_act:
                        nc.scalar.activation(v_sb[:, b, st, :, 0:64],
                                             ps[:, :, :], AF.Copy,
                                             scale=1.0 / WSC)
                    else:
                        nc.vector.tensor_scalar_mul(v_sb[:, b, st, :, 0:64],
                                                    ps[:, :, :], 1.0 / WSC)
                return [emit]

            def proj_unit(pool, b, ot, ch, outT_sb, nm, on_act=False):
                """Output projection for one (128 out-dims, 512 tokens)
                block: 1 matmul -> stage copy (DVE, or ACT when it has
                slack) -> DMA store."""
                def emit():
                    ps = pool.tile([128, 512], f32, tag=pool_tag[id(pool)],
                                   name=f"pj{nm}")
                    nc.tensor.matmul(
                        ps[:, :],
                        w2_sb[:, ot * 128:(ot + 1) * 128],
                        outT_sb[:, ch, :],
                        start=True, stop=True,
                    )
                    stage = stage_pool.tile([128, 512], f16, tag="st",
                                            name=f"st{nm}")
                    if on_act:
                        nc.scalar.copy(stage[:, :], ps[:, :])
                    else:
                        nc.vector.tensor_copy(stage[:, :], ps[:, :])
                    nc.sync.dma_start(
                        out_d[ot * 128:(ot + 1) * 128,
                              b * 2048 + ch * 512:b * 2048 + (ch + 1) * 512],
                        stage[:, :],
                    )
                return [emit]

            with tc.tile_pool(name="wvps", bufs=2, space="PSUM") as W:
                pool_tag = {id(W): "wv"}
                # PE clock-ramp bridge: keep the PE continuously busy from
                # t~0.8us until the first q8 chunk lands (~5.8us), so the
                # first projection runs at full clock (idle resets the ramp)
                wps = W.tile([128, 128], f32, tag="wv", name="wps")
                for i in range(N_WARM):
                    nc.tensor.matmul(wps[:, :], warm_mm[:, :], warm_mm[:, :],
                                     start=True, stop=True)

                # ---- phase 1: minimal solid pre-work, with the copy chain
                # hand-ordered so sc(tt0)'s inputs (K_hi, K_lo, Q8 planes)
                # complete as early as possible; Q_lo (only needed by the
                # first EXACT tile, tt=1) trails ----
                kps = W.tile([128, 512], f32, tag="wv", name="qkk00")
                qps = W.tile([128, 512], f32, tag="wv", name="qkq00")
                for m, ps in ((1, kps), (0, qps)):
                    for ti, (q8, w8) in enumerate(
                            ((q8hi_sb, w8hi_sb), (q8lo_sb, w8hi_sb),
                             (q8hi_sb, w8lo_sb))):
                        for i in range(4):
                            nc.tensor.matmul(
                                ps[:, :],
                                w8[:, 2 * i:2 * i + 2,
                                   m * 128:(m + 1) * 128],
                                q8[:, 2 * i:2 * i + 2, 0:512],
                                start=(ti == 0 and i == 0),
                                stop=(ti == 2 and i == 3),
                                perf_mode=DR,
                            )
                # sc(b0,ch0,tt0) needs only K_hi + the Q8 plane: its other
                # DR operand planes were zero-memset above, and the dup /
                # K_lo / Q_lo writes are emitted AFTER sc0 (WAR-ordered
                # behind its read) just inside the psum-pool block below
                kt0 = ktmp_pool.tile([128, 512], f16, tag="kt", name="kt00")
                nc.vector.tensor_scalar(kt0[:, :], kps[:, :],
                                        1.0 / WSC, qkb_sb[:, 1:2],
                                        AluOp.mult, AluOp.add)
                nc.vector.tensor_copy(qk8_sb[:, 1, 0, 0, 0:512], kt0[:, :])
                nc.vector.tensor_scalar(qk8_sb[:, 0, 0, 0, 0:512], qps[:, :],
                                        1.0 / WSC, qkb_sb[:, 0:1],
                                        AluOp.mult, AluOp.add)
                for part in vt_unit(W, 0, 0, "v00"):
                    part()

                # ---- weave schedule -------------------------------------
                sched = {}

                def assign(slots, parts):
                    assert len(slots) >= len(parts), (len(slots), len(parts))
                    for s, p in zip(slots, parts):
                        sched.setdefault(s, []).append(p)

                def qk_slots(b, ch, t1, t2, t3, t4):
                    return [(b, ch, t1), (b, ch, t2), (b, ch, t3), (b, ch, t4)]

                # b0 ch0: K(scc1-3) ahead of score deadlines (tt=4*scc),
                # V^T(st1-15) ahead of pv deadlines (tt=st)
                assign(qk_slots(0, 0, 1, 1, 2, 3), qk_unit(W, 0, 1, 1, "k01", skip_wlo=True))
                assign(qk_slots(0, 0, 5, 5, 6, 7), qk_unit(W, 0, 1, 2, "k02", skip_wlo=True))
                assign(qk_slots(0, 0, 9, 9, 10, 11), qk_unit(W, 0, 1, 3, "k03", skip_wlo=True))
                for st in range(1, 16):
                    assign([(0, 0, st)], vt_unit(W, 0, st, f"v0{st}"))
                # Q chunks for b0 ch1-3 (due at the start of their chunk)
                assign(qk_slots(0, 0, 13, 13, 14, 15), qk_unit(W, 0, 0, 1, "q01"))
                assign(qk_slots(0, 1, 1, 1, 2, 3), qk_unit(W, 0, 0, 2, "q02"))
                assign(qk_slots(0, 2, 1, 1, 2, 3), qk_unit(W, 0, 0, 3, "q03"))
                # b1 prep spread across b0 ch2/ch3
                assign(qk_slots(0, 2, 5, 5, 6, 7), qk_unit(W, 1, 1, 0, "k10"))
                assign(qk_slots(0, 2, 9, 9, 10, 11), qk_unit(W, 1, 0, 0, "q10"))
                for st in range(0, 4):
                    assign([(0, 2, 12 + st)], vt_unit(W, 1, st, f"v1{st}"))
                assign(qk_slots(0, 3, 1, 1, 2, 3), qk_unit(W, 1, 1, 1, "k11"))
                for st in range(4, 8):
                    assign([(0, 3, 4 + st)], vt_unit(W, 1, st, f"v1{st}"))
                # b1 ch0: K(scc2,3) due tt 8,12; V^T(st8-15) due tt 8-15
                assign(qk_slots(1, 0, 1, 1, 2, 3), qk_unit(W, 1, 1, 2, "k12"))
                assign(qk_slots(1, 0, 5, 5, 6, 7), qk_unit(W, 1, 1, 3, "k13"))
                for st in range(8, 16):
                    assign([(1, 0, st)], vt_unit(W, 1, st, f"v1{st}"))
                assign(qk_slots(1, 0, 9, 9, 10, 11), qk_unit(W, 1, 0, 1, "q11"))
                assign(qk_slots(1, 1, 1, 1, 2, 3), qk_unit(W, 1, 0, 2, "q12"))
                assign(qk_slots(1, 2, 1, 1, 2, 3), qk_unit(W, 1, 0, 3, "q13"))

                # proj slots per just-finished global chunk g = 4*b + ch
                PROJ_SLOTS = {
                    0: [(0, 1, t) for t in range(6, 14)],
                    1: [(0, 3, t) for t in (4, 5, 6, 7, 12, 13, 14, 15)],
                    2: [(1, 1, t) for t in range(4, 12)],
                    3: [(1, 1, t) for t in range(12, 16)]
                       + [(1, 2, t) for t in range(4, 8)],
                    4: [(1, 2, t) for t in range(8, 16)],
                    5: [(1, 3, t) for t in range(1, 9)],
                    6: [(1, 3, t) for t in range(9, 16)] + [(1, 3, 15)],
                }

                # ---- attention + normalize + woven projections ----------
                # pv runs TWO iterations behind exp so the PE never waits
                # on an in-flight exp; each chunk's last two pv tiles, its
                # normalize, and its proj assignment are deferred into the
                # next chunk's first iterations (so the in-order PE queue
                # never blocks on the chunk's final exp at the boundary)
                outT_tiles = {}

                def finish_chunk(pend, tt_step):
                    fb, fch, fpv, felog, foutT = pend
                    if tt_step == 0:
                        pe, ptt = felog[14]
                        for h in range(HPC):
                            nc.tensor.matmul(
                                fpv[h][:, :], v_sb[:, fb, 14, h, :],
                                pe[:, h, :], start=False, stop=False)
                        return
                    pe, ptt = felog[15]
                    for h in range(HPC):
                        nc.tensor.matmul(
                            fpv[h][:, :], v_sb[:, fb, 15, h, :],
                            pe[:, h, :], start=False, stop=True)
                    for h in range(HPC):
                        pvs = pvsb_pool.tile([65, 512], f32, tag="pvs",
                                             name=f"pvs{fb}{fch}{h}")
                        nc.vector.tensor_copy(pvs[:, :], fpv[h][:, :])
                        recip = recip_pool.tile([1, 512], f32, tag="rc",
                                                name=f"rc{fb}{fch}{h}")
                        nc.vector.reciprocal(recip[:, :], pvs[64:65, :])
                        rep = rep_pool.tile([64, 512], f32, tag="rp",
                                            name=f"rp{fb}{fch}{h}")
                        nc.gpsimd.partition_broadcast(rep[:, :], recip[:, :])
                        nc.vector.tensor_mul(
                            foutT[h * 64:(h + 1) * 64, fch, :],
                            pvs[0:64, :], rep[:, :])
                    g = 4 * fb + fch
                    if g in PROJ_SLOTS:
                        parts = []
                        for ot in range(8):
                            parts += proj_unit(W, fb, ot, fch, foutT,
                                               f"p{fb}{fch}_{ot}")
                        assign(PROJ_SLOTS[g], parts)

                with tc.tile_pool(name="scps", bufs=2, space="PSUM") as scps, \
                     tc.tile_pool(name="pvps", bufs=2, space="PSUM") as pvps:
                    pool_tag[id(pvps)] = "pv"
                    # sc(b0,ch0,tt0) emitted FIRST: K_hi @ Q8 only (other
                    # planes zeroed); the dup / K_lo / Q_lo writes follow
                    # and are WAR-ordered behind its read
                    sc0 = scps.tile([128, 2, 512], f32, tag="sc",
                                    name="sc0_0_0")
                    for h in range(HPC):
                        nc.tensor.matmul(
                            sc0[:, h, :],
                            qk8_sb[h * 64:h * 64 + 64, 1, 0, :, 0:128],
                            qk8_sb[h * 64:h * 64 + 64, 0, 0, :, 0:512],
                            start=True, stop=True, perf_mode=DR,
                        )
                    e0 = exp_pool.tile([128, 2, 512], f16, tag="exp",
                                       name="e0_0_0")
                    nc.scalar.activation(e0[:, :, :], sc0[:, :, :],
                                         AF.Exp, scale=0.125)
                    nc.vector.tensor_copy(qk8_sb[:, 0, 0, 1, 0:512],
                                          qk8_sb[:, 0, 0, 0, 0:512])
                    nc.vector.tensor_sub(qk8_sb[:, 1, 0, 1, 0:512],
                                         kt0[:, :],
                                         qk8_sb[:, 1, 0, 0, 0:512])
                    qt0 = ktmp_pool.tile([128, 512], f16, tag="kt",
                                         name="qt00")
                    nc.vector.tensor_scalar(qt0[:, :], qps[:, :],
                                            1.0 / WSC, qkb_sb[:, 0:1],
                                            AluOp.mult, AluOp.add)
                    nc.vector.tensor_sub(qlo8_sb[:, 0, 0, 0:512], qt0[:, :],
                                         qk8_sb[:, 0, 0, 0, 0:512])
                    nc.vector.tensor_copy(qlo8_sb[:, 0, 1, 0:512],
                                          qlo8_sb[:, 0, 0, 0:512])
                    pending = None
                    for b in range(B):
                        outT_sb = outT_pool.tile([128, 4, 512], f16,
                                                 tag="outT", name=f"outT{b}")
                        outT_tiles[b] = outT_sb
                        for ch in range(4):
                            s0 = ch * 512
                            pv = None
                            elog = []
                            for tt in range(16):
                                if b == 0 and ch == 0 and tt == 0:
                                    elog.append((e0, 0))
                                    continue
                                t0 = tt * 128
                                sc = scps.tile([128, 2, 512], f32, tag="sc",
                                               name=f"sc{b}_{ch}_{tt}")
                                # EXACT_TT tiles get the K @ Q_lo correction
                                # -> exact scores there; the rest carry only
                                # the single Q8 quantization
                                exact = tt in EXACT_TT
                                for h in range(HPC):
                                    lo = h * 64
                                    hi = lo + 64
                                    nc.tensor.matmul(
                                        sc[:, h, :],
                                        qk8_sb[lo:hi, 1, b, :, t0:t0 + 128],
                                        qk8_sb[lo:hi, 0, b, :, s0:s0 + 512],
                                        start=True, stop=not exact,
                                        perf_mode=DR,
                                    )
                                    if exact:
                                        nc.tensor.matmul(
                                            sc[:, h, :],
                                            qk8_sb[lo:hi, 1, b, :,
                                                   t0:t0 + 128],
                                            qlo8_sb[lo:hi, b, :,
                                                    s0:s0 + 512],
                                            start=False, stop=True,
                                            perf_mode=DR,
                                        )
                                for u in sched.get((b, ch, tt), ()):
                                    u()
                                if tt < 2:
                                    if pending is not None:
                                        finish_chunk(pending, tt)
                                        if tt == 1:
                                            pending = None
                                else:
                                    if tt == 2:
                                        pv = [pvps.tile([65, 512], f32,
                                                        tag="pv",
                                                        name=f"pv{b}{ch}{h}")
                                              for h in range(HPC)]
                                    pe, ptt = elog[tt - 2]
                                    for h in range(HPC):
                                        nc.tensor.matmul(
                                            pv[h][:, :],
                                            v_sb[:, b, ptt, h, :],
                                            pe[:, h, :],
                                            start=(ptt == 0), stop=False,
                                        )
                                    if b == 1 and ch == 3 and tt == 15:
                                        # last chunk: pull pv(14) into the
                                        # loop (overlaps exp 15) so the
                                        # tail waits only on pv(15)
                                        pe14, _ = elog[14]
                                        for h in range(HPC):
                                            nc.tensor.matmul(
                                                pv[h][:, :],
                                                v_sb[:, b, 14, h, :],
                                                pe14[:, h, :],
                                                start=False, stop=False,
                                            )
                                e = exp_pool.tile([128, 2, 512], f16,
                                                  tag="exp",
                                                  name=f"e{b}_{ch}_{tt}")
                                nc.scalar.activation(e[:, :, :], sc[:, :, :],
                                                     AF.Exp, scale=0.125)
                                elog.append((e, tt))
                            pending = (b, ch, pv, elog, outT_sb)

                    # ---- tail: the final chunk's catch-up, then normalize
                    # and projection processed in TWO 256-column halves so
                    # every stage (DVE copies/recips/mults, Pool broadcasts,
                    # PE proj matmuls, stage copies, DMA) pipelines; a short
                    # warm run keeps the PE clock ramped through the wait.
                    fb, fch, fpv, felog, foutT = pending
                    for ptt in (15,):  # pv(14) already ran in the loop
                        pe, _ = felog[ptt]
                        for h in range(HPC):
                            nc.tensor.matmul(
                                fpv[h][:, :], v_sb[:, fb, ptt, h, :],
                                pe[:, h, :], start=False, stop=(ptt == 15))
                    wps2 = W.tile([128, 128], f32, tag="wv", name="wps2")
                    for i in range(40):
                        nc.tensor.matmul(wps2[:, :], warm_mm[:, :],
                                         warm_mm[:, :], start=True, stop=True)
                    for hf in range(2):
                        cs = slice(hf * 256, (hf + 1) * 256)
                        for h in range(HPC):
                            pvs = pvsb_pool.tile([65, 256], f32, tag="pvs",
                                                 name=f"pvsT{h}{hf}")
                            if h == 0:  # ACT is idle post-exp: split lanes
                                nc.scalar.copy(pvs[:, :], fpv[h][:, cs])
                            else:
                                nc.vector.tensor_copy(pvs[:, :],
                                                      fpv[h][:, cs])
                            recip = recip_pool.tile([1, 256], f32, tag="rc",
                                                    name=f"rcT{h}{hf}")
                            nc.vector.reciprocal(recip[:, :], pvs[64:65, :])
                            rep = rep_pool.tile([64, 256], f32, tag="rp",
                                                name=f"rpT{h}{hf}")
                            nc.gpsimd.partition_broadcast(rep[:, :],
                                                          recip[:, :])
                            nc.vector.tensor_mul(
                                foutT[h * 64:(h + 1) * 64, fch, cs],
                                pvs[0:64, :], rep[:, :])
                    for hf in range(2):
                        cs = slice(hf * 256, (hf + 1) * 256)
                        c0 = fb * 2048 + fch * 512 + hf * 256
                        for op in range(4):  # pairs of ot blocks
                            # each pair: one 1-bank psum tile, one copy,
                            # one DMA; psums rotate over three pools (the
                            # idle scps banks included) for a 6-deep pipe
                            pool = (scps, W, pvps)[(hf * 4 + op) % 3]
                            if pool is scps:
                                tl = scps.tile([128, 2, 512], f32,
                                               tag="sc",
                                               name=f"pjt{op}{hf}")
                                sub = lambda j: tl[:, j, 0:256]
                                pr = tl[:, :, 0:256]
                            else:
                                tl = pool.tile([128, 2, 256], f32,
                                               tag=pool_tag[id(pool)],
                                               name=f"pjt{op}{hf}")
                                sub = lambda j: tl[:, j, :]
                                pr = tl[:, :, :]
                            for j in range(2):
                                nc.tensor.matmul(
                                    sub(j),
                                    w2_sb[:, (2 * op + j) * 128:
                                          (2 * op + j + 1) * 128],
                                    foutT[:, fch, cs],
                                    start=True, stop=True,
                                )
                            stage = stage_pool.tile([128, 2, 256], f16,
                                                    tag="st",
                                                    name=f"stt{op}{hf}")
                            # DVE still owes the norm chain; give ACT the
                            # larger share of the pair copies
                            if hf * 4 + op < 5:
                                nc.scalar.copy(stage[:, :, :], pr)
                            else:
                                nc.vector.tensor_copy(stage[:, :, :], pr)
                            # spread the tail stores across three DGE
                            # queues: the SP sequencer's ~650ns/dispatch
                            # would otherwise pace them
                            # DVE-copied pairs store via the ACT DGE queue:
                            # those dispatches sit after all ACT copies in
                            # its FIFO, running parallel to SP's dispatches
                            dq = nc.scalar if hf * 4 + op >= 5 else nc.sync
                            dq.dma_start(
                                out_d[2 * op * 128:(2 * op + 2) * 128,
                                      c0:c0 + 256].rearrange(
                                          "(n p) m -> p n m", p=128),
                                stage[:, :, :],
                            )
    nc.compile()
    return nc


def _get_nc():
    if "nc" not in _COMPILED:
        _COMPILED["nc"] = _build()
    return _COMPILED["nc"]


def _prep_inputs(q, in_w, qkv_bias, out_w):
    import ml_dtypes
    f16 = np.float16
    f8 = ml_dtypes.float8_e4m3
    F = np.float32
    qT = np.ascontiguousarray(q.transpose(2, 0, 1).reshape(D, BS))
    q8hi = qT.astype(f8)
    q8lo = (qT - q8hi.astype(F)).astype(f8)

    def warr(wT, cols):  # [D, cols] -> scaled fp8 split, [128, 8*cols]
        ws = wT * WSC
        hi = ws.astype(f8)
        lo = (ws - hi.astype(F)).astype(f8)

        def pack(a):
            return np.ascontiguousarray(
                a.reshape(8, 128, cols).transpose(1, 0, 2).reshape(128, -1))
        return pack(hi), pack(lo)

    maps = []
    for c in range(NCORES):
        r = slice(128 * c, 128 * (c + 1))
        wq, wk, wv = in_w[0:D][r], in_w[D:2 * D][r], in_w[2 * D:3 * D][r]
        wqk = np.ascontiguousarray(np.concatenate([wq, wk], 0).T)  # [D, 256]
        w8hi, w8lo = warr(wqk, 256)
        wv8hi, wv8lo = warr(np.ascontiguousarray(wv.T), 128)
        qkb = np.stack([qkv_bias[0:D][r], qkv_bias[D:2 * D][r]],
                       axis=1).astype(F)  # [128, 2]
        maps.append({
            "q8hi": q8hi,
            "q8lo": q8lo,
            "w8hi": w8hi,
            "w8lo": w8lo,
            "wv8hi": wv8hi,
            "wv8lo": wv8lo,
            "w2": np.ascontiguousarray(out_w[:, r].T).astype(f16),
            "qkb": np.ascontiguousarray(qkb),
            "vb": np.ascontiguousarray(
                (qkv_bias[2 * D:3 * D][r] * WSC)[None, :]).astype(f16),
        })
    return maps


def kernel(q, k, v, in_w, qkv_bias, out_w, out_b, _trace=False):
    from concourse.bass_utils import run_bass_kernel_spmd

    q = np.asarray(q, dtype=np.float32)
    in_w = np.asarray(in_w, dtype=np.float32)
    qkv_bias = np.asarray(qkv_bias, dtype=np.float32)
    out_w = np.asarray(out_w, dtype=np.float32)
    out_b = np.asarray(out_b, dtype=np.float32)

    nc = _get_nc()
    in_maps = _prep_inputs(q, in_w, qkv_bias, out_w)

    res = run_bass_kernel_spmd(
        nc, in_maps, core_ids=list(range(NCORES)), trace=_trace,
    )
    total = np.zeros((D, BS), dtype=np.float32)
    for c in range(NCORES):
        total += res.results[c]["partial"].astype(np.float32)
    net = total.T + out_b[None, :]
    out = net.reshape(B, S, D).astype(np.float32)
    if _trace:
        return out, res
    return out
